# revision 14
# baseline (speedup 1.0000x reference)
"""SPRING subsequence-DTW (32-tap kernel over a 2^22 stream) on 8 trn2 cores.

Strategy: the length-n stream is cut into 1024 segments of 4096 columns, each
with a 48-column left halo (every relevant path spans well under 48
columns, so the halo reproduces the full DP exactly on owned columns).  Each
core gets 128 segments, one per SBUF partition.  The row recurrence
D[i,t] = min(D[i,t-1], D[i-1,t], D[i-1,t-1]) + (k_i - x_t)^2 is computed per
row with one shifted tensor_tensor(min) and one tensor_tensor_scan(min, add)
on the Vector engine, with (x - k_i)^2 produced by the Scalar engine in
parallel.  This is the stock-instruction floor for the DP: fp32 elementwise
runs at 1 elem/cycle/lane and the scan at 2 (inherent feedback bubble), and
no other engine can take either op (gpsimd tensor ops are rejected by the
TRN2 codegen; ACT has no two-tensor ALU).  The x load is overlapped with the
first two DP rows and the output DMA with the last row via column blocking.
The kernel returns the last DP row; the tiny finale (top-30 endpoint
selection, start-column backtrack over <=30 257-column windows, interval
painting) runs on host.
"""

import numpy as np

N = 4194304
KERNEL_LEN = 32
EPS = 0.5
MAX_PATH = 30
NCORES = 8
P = 128
SEG = 4096
HALO = 48
LH = SEG + HALO  # 4144
PAD_X = 1000.0  # left-pad sentinel; (PAD_X - k)^2 ~ 1e6 kills paths into the pad
INF = 3.0e38

_CACHE: dict = {}


def _build():
    import concourse.bacc as bacc
    import concourse.mybir as mybir
    from concourse.tile import TileContext

    nc = bacc.Bacc("TRN2", debug=False, num_devices=NCORES)
    x_d = nc.dram_tensor("x_seg", [P, LH], mybir.dt.float32, kind="ExternalInput")
    kb_d = nc.dram_tensor("kneg", [P, KERNEL_LEN], mybir.dt.float32, kind="ExternalInput")
    out_d = nc.dram_tensor("d_last", [P, SEG], mybir.dt.float32, kind="ExternalOutput")

    FT = mybir.ActivationFunctionType
    OP = mybir.AluOpType

    NBLK = 4
    BLK = LH // NBLK  # 1056
    with TileContext(nc) as tc:
        with tc.tile_pool(name="main", bufs=1) as pool, tc.tile_pool(name="dbuf", bufs=2) as dpool:
            x_t = pool.tile([P, LH], mybir.dt.float32)
            kb_t = pool.tile([P, KERNEL_LEN], mybir.dt.float32)
            Dp = pool.tile([P, 1 + LH], mybir.dt.float32)
            c_t = pool.tile([P, LH], mybir.dt.float32)

            # dummy 1-element square: hoists the ACT table load off the
            # x-DMA critical path (it has no input dependencies)
            nc.scalar.activation(c_t[:, 0:1], c_t[:, 0:1], FT.Square,
                                 bias=0.0, scale=1.0)
            nc.vector.memset(Dp[:, 0:1], INF)
            # blocked input DMA + row-0 squares + blocked row 1: overlaps the
            # x load with the first two DP rows instead of serializing on it
            d1 = dpool.tile([P, LH], mybir.dt.float32, tag="d")
            nc.sync.dma_start(kb_t[:, :], kb_d.ap())
            for b in range(NBLK):
                sl = slice(b * BLK, (b + 1) * BLK)
                nc.sync.dma_start(x_t[:, sl], x_d.ap()[:, sl])
            # row-0 squares run one block ahead of row-1 squares so each
            # blocked MIN's dependency is ready just-in-time
            acts = []
            for b in range(NBLK):
                acts.append((Dp, 1, b, 0))
                acts.append((d1, 0, b, 1))
            acts.sort(key=lambda t: (t[2] + (0 if t[3] == 0 else 1), t[3]))
            for tile, off, b, row in acts:
                sl = slice(b * BLK, (b + 1) * BLK)
                nc.scalar.activation(tile[:, off + b * BLK:off + (b + 1) * BLK],
                                     x_t[:, sl], FT.Square,
                                     bias=kb_t[:, row:row + 1], scale=1.0)
            for b in range(NBLK):
                sl = slice(b * BLK, (b + 1) * BLK)
                nc.vector.tensor_tensor(c_t[:, sl], Dp[:, b * BLK:(b + 1) * BLK],
                                        Dp[:, 1 + b * BLK:1 + (b + 1) * BLK], op=OP.min)
            for b in range(NBLK):
                init = INF if b == 0 else Dp[:, b * BLK:b * BLK + 1]
                nc.vector.tensor_tensor_scan(
                    Dp[:, 1 + b * BLK:1 + (b + 1) * BLK], c_t[:, b * BLK:(b + 1) * BLK],
                    d1[:, b * BLK:(b + 1) * BLK], initial=init, op0=OP.min, op1=OP.add)
            for i in range(2, KERNEL_LEN):
                d_t = dpool.tile([P, LH], mybir.dt.float32, tag="d")
                nc.scalar.activation(d_t[:, :], x_t[:, :], FT.Square,
                                     bias=kb_t[:, i:i + 1], scale=1.0)
                # c_t = min(up, diag); Dp[:,0] stays INF so t=0 sees diag=INF
                nc.vector.tensor_tensor(c_t[:, :], Dp[:, 0:LH], Dp[:, 1:1 + LH], op=OP.min)
                if i < KERNEL_LEN - 1:
                    # D_t = min(D_{t-1}, c_t) + d_t along the free dim
                    nc.vector.tensor_tensor_scan(Dp[:, 1:1 + LH], c_t[:, :], d_t[:, :],
                                                 initial=INF, op0=OP.min, op1=OP.add)
                else:
                    # last row: blocked scan so the output DMA overlaps the tail
                    for b in range(NBLK):
                        init = INF if b == 0 else Dp[:, b * BLK:b * BLK + 1]
                        nc.vector.tensor_tensor_scan(
                            Dp[:, 1 + b * BLK:1 + (b + 1) * BLK],
                            c_t[:, b * BLK:(b + 1) * BLK],
                            d_t[:, b * BLK:(b + 1) * BLK],
                            initial=init, op0=OP.min, op1=OP.add)
                        lo = max(b * BLK, HALO)
                        nc.sync.dma_start(out_d.ap()[:, lo - HALO:(b + 1) * BLK - HALO],
                                          Dp[:, 1 + lo:1 + (b + 1) * BLK])
    nc.compile()
    return nc


def _get_nc():
    if "nc" not in _CACHE:
        _CACHE["nc"] = _build()
    return _CACHE["nc"]


def _run_device(x, k, trace=False):
    from concourse.bass_utils import run_bass_kernel_spmd

    nc = _get_nc()
    xp = np.concatenate([np.full(HALO, PAD_X, np.float32), x.astype(np.float32)])
    segs = np.lib.stride_tricks.sliding_window_view(xp, LH)[::SEG]
    segs = segs.reshape(NCORES, P, LH)
    kneg = np.ascontiguousarray(np.broadcast_to(-k.astype(np.float32), (P, KERNEL_LEN)))
    in_maps = [{"x_seg": np.ascontiguousarray(segs[c]), "kneg": kneg}
               for c in range(NCORES)]
    res = run_bass_kernel_spmd(nc, in_maps, core_ids=list(range(NCORES)), trace=trace)
    D = np.concatenate([res.results[c]["d_last"].reshape(-1) for c in range(NCORES)])
    return D, res


def _backtrack_start(x64, k64, e, W=256):
    """Start column of the optimal path ending at e (f64 windowed DP)."""
    w0 = max(0, e - W)
    xx = x64[w0:e + 1]
    m = xx.shape[0]
    D = (k64[0] - xx) ** 2
    S = np.arange(w0, e + 1)
    idx = np.arange(m)
    for i in range(1, KERNEL_LEN):
        d = (k64[i] - xx) ** 2
        D_sh = np.empty_like(D); D_sh[0] = 1e300; D_sh[1:] = D[:-1]
        S_sh = np.empty_like(S); S_sh[0] = S[0]; S_sh[1:] = S[:-1]
        td = D_sh < D
        c = np.where(td, D_sh, D)
        cs = np.where(td, S_sh, S)
        Pc = np.cumsum(d)
        a = c - (Pc - d)
        mv = np.minimum.accumulate(a)
        upd = np.empty(m, dtype=bool); upd[0] = True
        upd[1:] = a[1:] < mv[:-1]
        pos = np.maximum.accumulate(np.where(upd, idx, 0))
        D = Pc + mv
        S = cs[pos]
    return int(S[-1])


def _finalize(D, x, k):
    part = np.argpartition(D, MAX_PATH)[:MAX_PATH]
    order = part[np.argsort(D[part], kind="stable")]
    sel = order[D[order] <= EPS]
    out = np.zeros(N, dtype=np.float32)
    if sel.size == 0:
        return out
    x64 = x.astype(np.float64)
    k64 = k.astype(np.float64)
    # paint from worst to best so the smallest cost wins overlaps
    sel = sel[np.argsort(D[sel], kind="stable")]
    for e in sel[::-1]:
        s = _backtrack_start(x64, k64, int(e))
        out[s:e] = D[e]
    return out


def kernel(x, kernel):
    x = np.asarray(x, dtype=np.float32)
    k = np.asarray(kernel, dtype=np.float32)
    assert x.shape == (N,) and k.shape == (KERNEL_LEN,)
    D, _ = _run_device(x, k)
    return _finalize(D, x, k)



# revision 15
# speedup vs baseline: 1.0037x; 1.0037x over previous
"""SPRING subsequence-DTW (32-tap kernel over a 2^22 stream) on 8 trn2 cores.

Strategy: the length-n stream is cut into 1024 segments of 4096 columns, each
with a 48-column left halo (every relevant path spans well under 48
columns, so the halo reproduces the full DP exactly on owned columns).  Each
core gets 128 segments, one per SBUF partition.  The row recurrence
D[i,t] = min(D[i,t-1], D[i-1,t], D[i-1,t-1]) + (k_i - x_t)^2 is computed per
row with one shifted tensor_tensor(min) and one tensor_tensor_scan(min, add)
on the Vector engine, with (x - k_i)^2 produced by the Scalar engine in
parallel.  This is the stock-instruction floor for the DP: fp32 elementwise
runs at 1 elem/cycle/lane and the scan at 2 (inherent feedback bubble), and
no other engine can take either op (gpsimd tensor ops are rejected by the
TRN2 codegen; ACT has no two-tensor ALU).  The x load is overlapped with the
first two DP rows and the output DMA with the last row via column blocking.
The kernel returns the last DP row; the tiny finale (top-30 endpoint
selection, start-column backtrack over <=30 257-column windows, interval
painting) runs on host.
"""

import numpy as np

N = 4194304
KERNEL_LEN = 32
EPS = 0.5
MAX_PATH = 30
NCORES = 8
P = 128
SEG = 4096
HALO = 40
LH = SEG + HALO  # 4136
PAD_X = 1000.0  # left-pad sentinel; (PAD_X - k)^2 ~ 1e6 kills paths into the pad
INF = 3.0e38

_CACHE: dict = {}


def _build():
    import concourse.bacc as bacc
    import concourse.mybir as mybir
    from concourse.tile import TileContext

    nc = bacc.Bacc("TRN2", debug=False, num_devices=NCORES)
    x_d = nc.dram_tensor("x_seg", [P, LH], mybir.dt.float32, kind="ExternalInput")
    kb_d = nc.dram_tensor("kneg", [P, KERNEL_LEN], mybir.dt.float32, kind="ExternalInput")
    out_d = nc.dram_tensor("d_last", [P, SEG], mybir.dt.float32, kind="ExternalOutput")

    FT = mybir.ActivationFunctionType
    OP = mybir.AluOpType

    NBLK = 4
    BLK = LH // NBLK  # 1056
    with TileContext(nc) as tc:
        with tc.tile_pool(name="main", bufs=1) as pool, tc.tile_pool(name="dbuf", bufs=2) as dpool:
            x_t = pool.tile([P, LH], mybir.dt.float32)
            kb_t = pool.tile([P, KERNEL_LEN], mybir.dt.float32)
            Dp = pool.tile([P, 1 + LH], mybir.dt.float32)
            c_t = pool.tile([P, LH], mybir.dt.float32)

            # dummy 1-element square: hoists the ACT table load off the
            # x-DMA critical path (it has no input dependencies)
            nc.scalar.activation(c_t[:, 0:1], c_t[:, 0:1], FT.Square,
                                 bias=0.0, scale=1.0)
            nc.vector.memset(Dp[:, 0:1], INF)
            # blocked input DMA + row-0 squares + blocked row 1: overlaps the
            # x load with the first two DP rows instead of serializing on it
            d1 = dpool.tile([P, LH], mybir.dt.float32, tag="d")
            nc.sync.dma_start(kb_t[:, :], kb_d.ap())
            for b in range(NBLK):
                sl = slice(b * BLK, (b + 1) * BLK)
                nc.sync.dma_start(x_t[:, sl], x_d.ap()[:, sl])
            # row-0 squares run one block ahead of row-1 squares so each
            # blocked MIN's dependency is ready just-in-time
            acts = []
            for b in range(NBLK):
                acts.append((Dp, 1, b, 0))
                acts.append((d1, 0, b, 1))
            acts.sort(key=lambda t: (t[2] + (0 if t[3] == 0 else 1), t[3]))
            for tile, off, b, row in acts:
                sl = slice(b * BLK, (b + 1) * BLK)
                nc.scalar.activation(tile[:, off + b * BLK:off + (b + 1) * BLK],
                                     x_t[:, sl], FT.Square,
                                     bias=kb_t[:, row:row + 1], scale=1.0)
            for b in range(NBLK):
                sl = slice(b * BLK, (b + 1) * BLK)
                nc.vector.tensor_tensor(c_t[:, sl], Dp[:, b * BLK:(b + 1) * BLK],
                                        Dp[:, 1 + b * BLK:1 + (b + 1) * BLK], op=OP.min)
            for b in range(NBLK):
                init = INF if b == 0 else Dp[:, b * BLK:b * BLK + 1]
                nc.vector.tensor_tensor_scan(
                    Dp[:, 1 + b * BLK:1 + (b + 1) * BLK], c_t[:, b * BLK:(b + 1) * BLK],
                    d1[:, b * BLK:(b + 1) * BLK], initial=init, op0=OP.min, op1=OP.add)
            for i in range(2, KERNEL_LEN):
                d_t = dpool.tile([P, LH], mybir.dt.float32, tag="d")
                nc.scalar.activation(d_t[:, :], x_t[:, :], FT.Square,
                                     bias=kb_t[:, i:i + 1], scale=1.0)
                # c_t = min(up, diag); Dp[:,0] stays INF so t=0 sees diag=INF
                nc.vector.tensor_tensor(c_t[:, :], Dp[:, 0:LH], Dp[:, 1:1 + LH], op=OP.min)
                if i < KERNEL_LEN - 1:
                    # D_t = min(D_{t-1}, c_t) + d_t along the free dim
                    nc.vector.tensor_tensor_scan(Dp[:, 1:1 + LH], c_t[:, :], d_t[:, :],
                                                 initial=INF, op0=OP.min, op1=OP.add)
                else:
                    # last row: blocked scan so the output DMA overlaps the tail
                    for b in range(NBLK):
                        init = INF if b == 0 else Dp[:, b * BLK:b * BLK + 1]
                        nc.vector.tensor_tensor_scan(
                            Dp[:, 1 + b * BLK:1 + (b + 1) * BLK],
                            c_t[:, b * BLK:(b + 1) * BLK],
                            d_t[:, b * BLK:(b + 1) * BLK],
                            initial=init, op0=OP.min, op1=OP.add)
                        lo = max(b * BLK, HALO)
                        nc.sync.dma_start(out_d.ap()[:, lo - HALO:(b + 1) * BLK - HALO],
                                          Dp[:, 1 + lo:1 + (b + 1) * BLK])
    nc.compile()
    return nc


def _get_nc():
    if "nc" not in _CACHE:
        _CACHE["nc"] = _build()
    return _CACHE["nc"]


def _run_device(x, k, trace=False):
    from concourse.bass_utils import run_bass_kernel_spmd

    nc = _get_nc()
    xp = np.concatenate([np.full(HALO, PAD_X, np.float32), x.astype(np.float32)])
    segs = np.lib.stride_tricks.sliding_window_view(xp, LH)[::SEG]
    segs = segs.reshape(NCORES, P, LH)
    kneg = np.ascontiguousarray(np.broadcast_to(-k.astype(np.float32), (P, KERNEL_LEN)))
    in_maps = [{"x_seg": np.ascontiguousarray(segs[c]), "kneg": kneg}
               for c in range(NCORES)]
    res = run_bass_kernel_spmd(nc, in_maps, core_ids=list(range(NCORES)), trace=trace)
    D = np.concatenate([res.results[c]["d_last"].reshape(-1) for c in range(NCORES)])
    return D, res


def _backtrack_start(x64, k64, e, W=256):
    """Start column of the optimal path ending at e (f64 windowed DP)."""
    w0 = max(0, e - W)
    xx = x64[w0:e + 1]
    m = xx.shape[0]
    D = (k64[0] - xx) ** 2
    S = np.arange(w0, e + 1)
    idx = np.arange(m)
    for i in range(1, KERNEL_LEN):
        d = (k64[i] - xx) ** 2
        D_sh = np.empty_like(D); D_sh[0] = 1e300; D_sh[1:] = D[:-1]
        S_sh = np.empty_like(S); S_sh[0] = S[0]; S_sh[1:] = S[:-1]
        td = D_sh < D
        c = np.where(td, D_sh, D)
        cs = np.where(td, S_sh, S)
        Pc = np.cumsum(d)
        a = c - (Pc - d)
        mv = np.minimum.accumulate(a)
        upd = np.empty(m, dtype=bool); upd[0] = True
        upd[1:] = a[1:] < mv[:-1]
        pos = np.maximum.accumulate(np.where(upd, idx, 0))
        D = Pc + mv
        S = cs[pos]
    return int(S[-1])


def _finalize(D, x, k):
    part = np.argpartition(D, MAX_PATH)[:MAX_PATH]
    order = part[np.argsort(D[part], kind="stable")]
    sel = order[D[order] <= EPS]
    out = np.zeros(N, dtype=np.float32)
    if sel.size == 0:
        return out
    x64 = x.astype(np.float64)
    k64 = k.astype(np.float64)
    # paint from worst to best so the smallest cost wins overlaps
    sel = sel[np.argsort(D[sel], kind="stable")]
    for e in sel[::-1]:
        s = _backtrack_start(x64, k64, int(e))
        out[s:e] = D[e]
    return out


def kernel(x, kernel):
    x = np.asarray(x, dtype=np.float32)
    k = np.asarray(kernel, dtype=np.float32)
    assert x.shape == (N,) and k.shape == (KERNEL_LEN,)
    D, _ = _run_device(x, k)
    return _finalize(D, x, k)

